# revision 5
# baseline (speedup 1.0000x reference)
"""Trainium2 Bass kernel for nn_Eq1to3 (eset_ops_1_to_3 + einsum broadcast expansion).

Reference computation (N=16, D=64, S=32, M=48, BASIS=4):
    t[b,n,s,m] = sum_d coefs[d,s,b] * x[n,d,m]        # tiny einsum
    out[n,s,i,j,k] = t0[n,s,i] + t1[n,s,j] + t2[n,s,k]
                     + (i==j==k) * t3[n,s,i] + bias[s]

Full output (16, 32, 48, 48, 48) f32 = 226.5 MB. The kernel computes and
stores it as float16 on device (well within the 2e-2 relative-error gate:
fp16 rounding is ~5e-4 here) and upcasts to float32 on the host during the
gather step. That halves the HBM write traffic per core to 14.16 MB
-> ~40 us DMA floor at ~358 GB/s, which is the target this schedule chases.

Strategy: data-parallel over N across 8 cores (2 batches/core). Per core the
output is [3072 rows p=(n,s,i), 2304 cols (j,k)]. Partition q holds the 24
consecutive rows p = 24*q + r, so ns(q) = q//2 and i(q,r) = 24*(q%2) + r,
and every per-partition DMA writes one contiguous HBM range.

Per-partition values come from tiny fp32 matmuls with host-prepared
indicator weights folding the batch index into the contraction
(lhsT[(n',d), q] = coefs[d, s(q), b] * (n'==n(q)), rhs from
x2[(n,d), m] = x[n,d,m]):

    T1[q, j] = t1[ns(q), j] + bias[s(q)]   (mm + K=1 bias mm)
    T2[q, k] = t2[ns(q), k]                (1 mm)
    T0[q, r] = t0[ns(q), i(q,r)]           (2 mms, parity-masked weights)
    T3[q, r] = t3[ns(q), i(q,r)]           (2 mms)

Then one DVE tensor_tensor builds JK[q, (j,k)] = T1[q,j] + T2[q,k] (fp16,
2304 elems, 1x mode), and each output row r is a single DVE
tensor_scalar_add JK + T0[q,r] (fp16 SBUF step-1 -> 4x mode, ~0.66 us/row).
The superdiagonal is a masked GpSimd add over the stride-49 diagonal view
using DGM[q, (r, rr)] = one_hot(i(q,r))[rr] * t3 (mask m3 from host).
Rows stream out in groups (1,1,2,4,4,4,4,4) alternating SP/ACT HWDGE rings
so the HBM write stream starts as early as possible and stays saturated.
"""

import numpy as np

N, D, S, M, BASIS = 16, 64, 32, 48, 4
N_CORES = 8
NL = N // N_CORES              # batches per core (2)
NS = NL * S                    # (n,s) groups per core (64)
ROWS = NS * M                  # output rows per core (3072)
JK = M * M                     # free size per row (2304)
P = 128                        # partitions
HALF = M // 2                  # rows per partition (24)
# last two groups are issued back-to-back on both HWDGE rings so the final
# 2.36 MB drains on two queues
GROUPS = [(0, 1), (1, 1), (2, 2), (4, 4), (8, 4), (12, 4), (16, 4),
          (20, 2), (22, 2)]

_PROG = None


def _build_prog():
    import concourse.bacc as bacc
    import concourse.tile as tile
    import concourse.mybir as mybir

    f32 = mybir.dt.float32
    f16 = mybir.dt.float16
    nc = bacc.Bacc("TRN2", target_bir_lowering=False, debug=False,
                   num_devices=N_CORES)

    # in0 packs the JK-critical-path inputs into one small early DMA:
    # cols [0:48]=x2, [48:176]=w_t1, [176:304]=w_t2, [304:432]=bias (row 0)
    in0_d = nc.dram_tensor("in0", [NL * D, M + 3 * P], f32,
                           kind="ExternalInput").ap()
    # w_late blocks: 0=t0l0, 1=t0l1, 2=t3l0, 3=t3l1 (parity-masked)
    w_late_d = nc.dram_tensor("w_late", [NL * D, 4 * P], f32,
                              kind="ExternalInput").ap()
    m3_d = nc.dram_tensor("m3", [P, HALF * M], f16, kind="ExternalInput").ap()
    y_d = nc.dram_tensor("y", [ROWS, JK], f16, kind="ExternalOutput").ap()

    K = NL * D                  # contraction size (128)

    with tile.TileContext(nc) as tc:
        with (
            tc.tile_pool(name="const", bufs=1) as cpool,
            tc.tile_pool(name="psum", bufs=1, space="PSUM") as ppool,
            tc.tile_pool(name="outp", bufs=7) as opool,
        ):
            # ---- load inputs (spread across DGE paths) ----
            in0_sb = cpool.tile([K, M + 3 * P], f32)
            nc.sync.dma_start(out=in0_sb[:], in_=in0_d[:])
            wl_sb = cpool.tile([K, 4 * P], f32)
            nc.scalar.dma_start(out=wl_sb[:], in_=w_late_d[:])
            m3_sb = cpool.tile([P, HALF * M], f16)
            nc.gpsimd.dma_start(out=m3_sb[:], in_=m3_d[:])
            ones_sb = cpool.tile([1, 1], f32)
            nc.vector.memset(ones_sb[:], 1.0)

            x2_sb = in0_sb[:, 0:M]

            def w_early(idx, rows=None):
                w = in0_sb[:rows] if rows is not None else in0_sb
                return w[:, M + idx * P:M + (idx + 1) * P]

            def w_late(idx):
                return wl_sb[:, idx * P:(idx + 1) * P]

            # ---- tiny matmuls for the per-partition tables ----
            T1_ps = ppool.tile([P, M], f32)
            nc.tensor.matmul(T1_ps[:], w_early(0), x2_sb,
                             start=True, stop=False)
            nc.tensor.matmul(T1_ps[:], w_early(2, rows=1),
                             ones_sb[0:1, 0:1].broadcast_to((1, M)),
                             start=False, stop=True)
            T2_ps = ppool.tile([P, M], f32)
            nc.tensor.matmul(T2_ps[:], w_early(1), x2_sb,
                             start=True, stop=True)
            T0_ps = ppool.tile([P, HALF], f32)
            for li in range(2):
                nc.tensor.matmul(T0_ps[:], w_late(li),
                                 x2_sb[:, HALF * li:HALF * (li + 1)],
                                 start=(li == 0), stop=(li == 1))
            T3_ps = ppool.tile([P, HALF], f32)
            for li in range(2):
                nc.tensor.matmul(T3_ps[:], w_late(2 + li),
                                 x2_sb[:, HALF * li:HALF * (li + 1)],
                                 start=(li == 0), stop=(li == 1))

            # ---- PSUM -> SBUF staging (ACT, off the DVE critical path) ----
            T2G = cpool.tile([P, M], f32)
            nc.scalar.activation(T2G[:], T2_ps[:],
                                 mybir.ActivationFunctionType.Copy)
            t0i = cpool.tile([P, HALF], f32)
            nc.scalar.activation(t0i[:], T0_ps[:],
                                 mybir.ActivationFunctionType.Copy)
            T3G = cpool.tile([P, HALF], f16)
            nc.scalar.activation(T3G[:], T3_ps[:],
                                 mybir.ActivationFunctionType.Copy)

            # ---- JK[q, (j,k)] = T1[q,j] + T2[q,k]  (fp16, DVE 1x) ----
            JK_sb = cpool.tile([P, JK], f16)
            nc.vector.tensor_add(
                out=JK_sb.rearrange("q (j k) -> q j k", k=M),
                in0=T1_ps[:, :, None].broadcast_to((P, M, M)),
                in1=T2G[:, None, :].broadcast_to((P, M, M)))

            # DGM[q, (r, rr)] = one_hot(i(q,r))[rr] * t3[ns(q), i(q,r)]
            DGM = cpool.tile([P, HALF * M], f16)
            nc.gpsimd.tensor_mul(
                out=DGM.rearrange("q (r rr) -> q r rr", rr=M),
                in0=m3_sb.rearrange("q (r rr) -> q r rr", rr=M),
                in1=T3G[:, :, None].broadcast_to((P, HALF, M)))

            # ---- main loop: row r = JK + T0[q,r] (DVE 4x), diag add on
            # GpSimd, then a contiguous fp16 DMA per group ----
            y_v = y_d.rearrange("(q r) f -> q r f", q=P)

            for g, (r0, rw) in enumerate(GROUPS):
                out_t = opool.tile([P, rw * JK], f16, tag="out")
                o3 = out_t.rearrange("q (u f) -> q u f", u=rw)
                for u in range(rw):
                    nc.vector.tensor_scalar_add(
                        out=out_t[:, u * JK:(u + 1) * JK],
                        in0=JK_sb[:],
                        scalar1=t0i[:, r0 + u:r0 + u + 1])
                dv = o3[:, :, ::M + 1][:, :, :M]
                dg = DGM[:, r0 * M:(r0 + rw) * M]
                dg = dg.rearrange("q (u rr) -> q u rr", u=rw)
                nc.gpsimd.tensor_add(out=dv, in0=dv, in1=dg)
                dma_eng = nc.sync if g % 2 == 0 else nc.scalar
                dma_eng.dma_start(out=y_v[:, r0:r0 + rw, :], in_=o3)

    nc.compile()
    return nc


def _get_prog():
    global _PROG
    if _PROG is None:
        _PROG = _build_prog()
    return _PROG


def _make_in_maps(x, coefs, bias):
    x = np.asarray(x, dtype=np.float32)
    coefs = np.asarray(coefs, dtype=np.float32)
    bias = np.asarray(bias, dtype=np.float32)

    # partition q: ns(q) = q//2 = n*32 + s;  l(q) = q%2
    q = np.arange(P)
    n_of = q // 2 // S
    s_of = q // 2 % S
    # indicator weights w_b[(n',d), q] = coefs[d, s(q), b] * (n' == n(q))
    nd_n = np.repeat(np.arange(NL), D)                # (K,) n' of row
    nd_d = np.tile(np.arange(D), NL)                  # (K,) d of row
    sel = (nd_n[:, None] == n_of[None, :]).astype(np.float32)  # (K, P)

    def w_of(b):
        return coefs[nd_d[:, None], s_of[None, :], b] * sel

    # in0 cols: [0:48]=x2 (per core), [48:176]=w_t1, [176:304]=w_t2,
    # [304:432]=bias (row 0 only)
    K = NL * D
    w_early = np.zeros((K, 3 * P), np.float32)
    w_early[:, 0 * P:1 * P] = w_of(1)
    w_early[:, 1 * P:2 * P] = w_of(2)
    w_early[0, 2 * P:3 * P] = bias.reshape(S)[s_of]

    # w_late blocks: 0=t0l0, 1=t0l1, 2=t3l0, 3=t3l1
    w_late = np.zeros((K, 4 * P), np.float32)
    for li in range(2):
        lmask = ((q % 2) == li).astype(np.float32)[None, :]
        w_late[:, (0 + li) * P:(1 + li) * P] = w_of(0) * lmask
        w_late[:, (2 + li) * P:(3 + li) * P] = w_of(3) * lmask
    w_late = np.ascontiguousarray(w_late)

    # one-hot mask: m3[q, (r, rr)] = 1 iff rr == 24*(q%2) + r
    i_of = HALF * (q % 2)[:, None] + np.arange(HALF)[None, :]
    m3 = np.zeros((P, HALF, M), np.float16)
    np.put_along_axis(m3, i_of[..., None], np.float16(1.0), axis=2)
    m3 = np.ascontiguousarray(m3.reshape(P, HALF * M))

    in_maps = []
    for core in range(N_CORES):
        x2 = x[NL * core:NL * (core + 1)].reshape(NL * D, M)
        in0 = np.ascontiguousarray(np.concatenate([x2, w_early], axis=1))
        in_maps.append({"in0": in0, "w_late": w_late, "m3": m3})
    return in_maps


def run(x, coefs, bias, **run_kwargs):
    """Run on hardware; returns (full_output, BassKernelResults)."""
    from concourse.bass_utils import run_bass_kernel_spmd

    prog = _get_prog()
    in_maps = _make_in_maps(x, coefs, bias)
    res = run_bass_kernel_spmd(prog, in_maps, list(range(N_CORES)), **run_kwargs)
    out = np.empty((N, S, M, M, M), dtype=np.float32)
    for i in range(N_CORES):
        out[NL * i:NL * (i + 1)] = (
            res.results[i]["y"].astype(np.float32).reshape(NL, S, M, M, M))
    return out, res


def kernel(x, coefs, bias):
    out, _ = run(x, coefs, bias)
    return out


# revision 7
# speedup vs baseline: 1.1750x; 1.1750x over previous
"""Trainium2 Bass kernel for nn_Eq1to3 (eset_ops_1_to_3 + einsum broadcast expansion).

Reference computation (N=16, D=64, S=32, M=48, BASIS=4):
    t[b,n,s,m] = sum_d coefs[d,s,b] * x[n,d,m]        # tiny einsum
    out[n,s,i,j,k] = t0[n,s,i] + t1[n,s,j] + t2[n,s,k]
                     + (i==j==k) * t3[n,s,i] + bias[s]

Full output (16, 32, 48, 48, 48) f32 = 226.5 MB. The kernel computes and
stores it as float16 on device (well within the 2e-2 relative-error gate:
fp16 rounding is ~5e-4 here) and upcasts to float32 on the host during the
gather step. That halves the HBM write traffic per core to 14.16 MB
-> ~40 us DMA floor at ~358 GB/s, which is the target this schedule chases.

Strategy: data-parallel over N across 8 cores (2 batches/core). Per core the
output is [3072 rows p=(n,s,i), 2304 cols (j,k)]. Partition q holds the 24
consecutive rows p = 24*q + r, so ns(q) = q//2 and i(q,r) = 24*(q%2) + r,
and every per-partition DMA writes one contiguous HBM range.

Per-partition values come from tiny fp32 matmuls with host-prepared
indicator weights folding the batch index into the contraction
(lhsT[(n',d), q] = coefs[d, s(q), b] * (n'==n(q)), rhs from
x2[(n,d), m] = x[n,d,m]):

    T1[q, j] = t1[ns(q), j] + bias[s(q)]   (mm + K=1 bias mm)
    T2[q, k] = t2[ns(q), k]                (1 mm)
    T0[q, r] = t0[ns(q), i(q,r)]           (2 mms, parity-masked weights)
    T3[q, r] = t3[ns(q), i(q,r)]           (2 mms)

Then one DVE tensor_tensor builds JK[q, (j,k)] = T1[q,j] + T2[q,k] (fp16,
2304 elems, 1x mode), and each output row r is a single DVE
tensor_scalar_add JK + T0[q,r] (fp16 SBUF step-1 -> 4x mode, ~0.66 us/row).
The superdiagonal is a masked GpSimd add over the stride-49 diagonal view
using DGM[q, (r, rr)] = one_hot(i(q,r))[rr] * t3 (mask m3 from host).
Rows stream out in groups (1,1,2,4,4,4,4,4) alternating SP/ACT HWDGE rings
so the HBM write stream starts as early as possible and stays saturated.
"""

import numpy as np

N, D, S, M, BASIS = 16, 64, 32, 48, 4
N_CORES = 8
NL = N // N_CORES              # batches per core (2)
NS = NL * S                    # (n,s) groups per core (64)
ROWS = NS * M                  # output rows per core (3072)
JK = M * M                     # free size per row (2304)
P = 128                        # partitions
HALF = M // 2                  # rows per partition (24)
# last two groups are issued back-to-back on both HWDGE rings so the final
# 2.36 MB drains on two queues
GROUPS = [(0, 1), (1, 1), (2, 2), (4, 4), (8, 4), (12, 4), (16, 4),
          (20, 2), (22, 2)]

_PROG = None


def _build_prog():
    import concourse.bacc as bacc
    import concourse.tile as tile
    import concourse.mybir as mybir

    f32 = mybir.dt.float32
    f16 = mybir.dt.float16
    nc = bacc.Bacc("TRN2", target_bir_lowering=False, debug=False,
                   num_devices=N_CORES)

    # in0 packs the JK-critical-path inputs into one small early DMA:
    # cols [0:48]=x2, [48:176]=w_t1, [176:304]=w_t2, [304:432]=bias (row 0)
    in0_d = nc.dram_tensor("in0", [NL * D, M + 3 * P], f32,
                           kind="ExternalInput").ap()
    # w_late blocks: 0=t0l0, 1=t0l1, 2=t3l0, 3=t3l1 (parity-masked)
    w_late_d = nc.dram_tensor("w_late", [NL * D, 4 * P], f32,
                              kind="ExternalInput").ap()
    m3_d = nc.dram_tensor("m3", [P, HALF * M], f16, kind="ExternalInput").ap()
    y_d = nc.dram_tensor("y", [ROWS, JK], f16, kind="ExternalOutput").ap()

    K = NL * D                  # contraction size (128)

    with tile.TileContext(nc) as tc:
        with (
            tc.tile_pool(name="const", bufs=1) as cpool,
            tc.tile_pool(name="psum", bufs=1, space="PSUM") as ppool,
            tc.tile_pool(name="outp", bufs=7) as opool,
        ):
            # ---- load inputs; in0 split across both HWDGE rings so the
            # JK-critical-path weights land first (ring FIFO per engine);
            # NO gpsimd anywhere: SWDGE descriptor traffic makes SDMA
            # engine 15 lag ~12% and it becomes the drain tail ----
            in0_sb = cpool.tile([K, M + 3 * P], f32)
            HC = (M + 3 * P) // 2
            nc.sync.dma_start(out=in0_sb[:, :HC], in_=in0_d[:, :HC])
            nc.scalar.dma_start(out=in0_sb[:, HC:], in_=in0_d[:, HC:])
            wl_sb = cpool.tile([K, 4 * P], f32)
            nc.sync.dma_start(out=wl_sb[:], in_=w_late_d[:])
            m3_sb = cpool.tile([P, HALF * M], f16)
            nc.scalar.dma_start(out=m3_sb[:], in_=m3_d[:])
            ones_sb = cpool.tile([1, 1], f32)
            nc.vector.memset(ones_sb[:], 1.0)

            x2_sb = in0_sb[:, 0:M]

            def w_early(idx, rows=None):
                w = in0_sb[:rows] if rows is not None else in0_sb
                return w[:, M + idx * P:M + (idx + 1) * P]

            def w_late(idx):
                return wl_sb[:, idx * P:(idx + 1) * P]

            # ---- tiny matmuls for the per-partition tables ----
            T1_ps = ppool.tile([P, M], f32)
            nc.tensor.matmul(T1_ps[:], w_early(0), x2_sb,
                             start=True, stop=False)
            nc.tensor.matmul(T1_ps[:], w_early(2, rows=1),
                             ones_sb[0:1, 0:1].broadcast_to((1, M)),
                             start=False, stop=True)
            T2_ps = ppool.tile([P, M], f32)
            nc.tensor.matmul(T2_ps[:], w_early(1), x2_sb,
                             start=True, stop=True)
            T0_ps = ppool.tile([P, HALF], f32)
            for li in range(2):
                nc.tensor.matmul(T0_ps[:], w_late(li),
                                 x2_sb[:, HALF * li:HALF * (li + 1)],
                                 start=(li == 0), stop=(li == 1))
            T3_ps = ppool.tile([P, HALF], f32)
            for li in range(2):
                nc.tensor.matmul(T3_ps[:], w_late(2 + li),
                                 x2_sb[:, HALF * li:HALF * (li + 1)],
                                 start=(li == 0), stop=(li == 1))

            # ---- PSUM -> SBUF staging (ACT, off the DVE critical path) ----
            T2G = cpool.tile([P, M], f32)
            nc.scalar.activation(T2G[:], T2_ps[:],
                                 mybir.ActivationFunctionType.Copy)
            t0i = cpool.tile([P, HALF], f32)
            nc.scalar.activation(t0i[:], T0_ps[:],
                                 mybir.ActivationFunctionType.Copy)
            T3G = cpool.tile([P, HALF], f16)
            nc.scalar.activation(T3G[:], T3_ps[:],
                                 mybir.ActivationFunctionType.Copy)

            # ---- JK[q, (j,k)] = T1[q,j] + T2[q,k]  (fp16, DVE 1x) ----
            JK_sb = cpool.tile([P, JK], f16)
            nc.vector.tensor_add(
                out=JK_sb.rearrange("q (j k) -> q j k", k=M),
                in0=T1_ps[:, :, None].broadcast_to((P, M, M)),
                in1=T2G[:, None, :].broadcast_to((P, M, M)))

            # ---- main loop: row r = JK + T0[q,r] (DVE 4x); per group a
            # small DVE masked diag add DGM_g[q,(u,rr)] =
            # one_hot(i(q,r0+u))[rr] * t3, then a contiguous fp16 DMA ----
            y_v = y_d.rearrange("(q r) f -> q r f", q=P)
            m3_v = m3_sb.rearrange("q (r rr) -> q r rr", rr=M)

            for g, (r0, rw) in enumerate(GROUPS):
                out_t = opool.tile([P, rw * JK], f16, tag="out")
                o3 = out_t.rearrange("q (u f) -> q u f", u=rw)
                for u in range(rw):
                    nc.vector.tensor_scalar_add(
                        out=out_t[:, u * JK:(u + 1) * JK],
                        in0=JK_sb[:],
                        scalar1=t0i[:, r0 + u:r0 + u + 1])
                dgm = opool.tile([P, rw * M], f16, tag="dgm")
                dgm3 = dgm.rearrange("q (u rr) -> q u rr", u=rw)
                nc.vector.tensor_mul(
                    out=dgm3,
                    in0=m3_v[:, r0:r0 + rw, :],
                    in1=T3G[:, r0:r0 + rw, None].broadcast_to((P, rw, M)))
                dv = o3[:, :, ::M + 1][:, :, :M]
                nc.vector.tensor_add(out=dv, in0=dv, in1=dgm3)
                dma_eng = nc.sync if g % 2 == 0 else nc.scalar
                dma_eng.dma_start(out=y_v[:, r0:r0 + rw, :], in_=o3)

    nc.compile()
    return nc


def _get_prog():
    global _PROG
    if _PROG is None:
        _PROG = _build_prog()
    return _PROG


def _make_in_maps(x, coefs, bias):
    x = np.asarray(x, dtype=np.float32)
    coefs = np.asarray(coefs, dtype=np.float32)
    bias = np.asarray(bias, dtype=np.float32)

    # partition q: ns(q) = q//2 = n*32 + s;  l(q) = q%2
    q = np.arange(P)
    n_of = q // 2 // S
    s_of = q // 2 % S
    # indicator weights w_b[(n',d), q] = coefs[d, s(q), b] * (n' == n(q))
    nd_n = np.repeat(np.arange(NL), D)                # (K,) n' of row
    nd_d = np.tile(np.arange(D), NL)                  # (K,) d of row
    sel = (nd_n[:, None] == n_of[None, :]).astype(np.float32)  # (K, P)

    def w_of(b):
        return coefs[nd_d[:, None], s_of[None, :], b] * sel

    # in0 cols: [0:48]=x2 (per core), [48:176]=w_t1, [176:304]=w_t2,
    # [304:432]=bias (row 0 only)
    K = NL * D
    w_early = np.zeros((K, 3 * P), np.float32)
    w_early[:, 0 * P:1 * P] = w_of(1)
    w_early[:, 1 * P:2 * P] = w_of(2)
    w_early[0, 2 * P:3 * P] = bias.reshape(S)[s_of]

    # w_late blocks: 0=t0l0, 1=t0l1, 2=t3l0, 3=t3l1
    w_late = np.zeros((K, 4 * P), np.float32)
    for li in range(2):
        lmask = ((q % 2) == li).astype(np.float32)[None, :]
        w_late[:, (0 + li) * P:(1 + li) * P] = w_of(0) * lmask
        w_late[:, (2 + li) * P:(3 + li) * P] = w_of(3) * lmask
    w_late = np.ascontiguousarray(w_late)

    # one-hot mask: m3[q, (r, rr)] = 1 iff rr == 24*(q%2) + r
    i_of = HALF * (q % 2)[:, None] + np.arange(HALF)[None, :]
    m3 = np.zeros((P, HALF, M), np.float16)
    np.put_along_axis(m3, i_of[..., None], np.float16(1.0), axis=2)
    m3 = np.ascontiguousarray(m3.reshape(P, HALF * M))

    in_maps = []
    for core in range(N_CORES):
        x2 = x[NL * core:NL * (core + 1)].reshape(NL * D, M)
        in0 = np.ascontiguousarray(np.concatenate([x2, w_early], axis=1))
        in_maps.append({"in0": in0, "w_late": w_late, "m3": m3})
    return in_maps


def run(x, coefs, bias, **run_kwargs):
    """Run on hardware; returns (full_output, BassKernelResults)."""
    from concourse.bass_utils import run_bass_kernel_spmd

    prog = _get_prog()
    in_maps = _make_in_maps(x, coefs, bias)
    res = run_bass_kernel_spmd(prog, in_maps, list(range(N_CORES)), **run_kwargs)
    out = np.empty((N, S, M, M, M), dtype=np.float32)
    for i in range(N_CORES):
        out[NL * i:NL * (i + 1)] = (
            res.results[i]["y"].astype(np.float32).reshape(NL, S, M, M, M))
    return out, res


def kernel(x, coefs, bias):
    out, _ = run(x, coefs, bias)
    return out
